# revision 10
# baseline (speedup 1.0000x reference)
"""MoE (8 experts, top-2) Trainium2 kernel.

Strategy (expert-parallel, per sharding hint):
  - Host: router (softmax + top-2 + renorm)  [0.1% of FLOPs], then
    all-to-all dispatch done host-side: gather each expert's tokens into a
    fixed-capacity buffer (bf16).
  - Device (8 cores, 1 expert each): Y_e = GELU(X_e @ W1[e] + b1[e]) @ W2[e] + b2[e]
    All matmuls bf16 (full 1-row/cycle PE rate).  Weights are streamed
    from HBM exactly ONCE (vs once per token-chunk): X (bf16, 4.5MB) and
    the full hidden activation h (bf16, 17.8MB) stay resident in SBUF.
  - Host: weighted combine (the return all-to-all) back to [B, L, D].

Device schedule per core (capacity `cap` tokens, sub-chunks of <=512):
  Phase 1 (per ft in 32):  dma W1[:,ft];  per sub:
      ps1[f128, sub] = sum_dt W1t[dt].T @ X[dt, sub]      (8 mms, bf16)
      h[ft, sub] = Gelu(ps1 + b1[ft])                     (ScalarE -> bf16)
  Phase 2 (per dt2 in 8):  dma W2[:,dt2];  per sub:
      ps2[d128, sub] = sum_ft W2t[ft].T @ h[ft, sub]      (32 mms, bf16)
      y[dt2, sub] = ps2 + b2[dt2]                         (DVE -> bf16)

PSUM: ps1 pool 4 banks + ps2 pool 4 banks = 8.  PE never waits on PSUM.
Compute floor: 2*cap*NDT*NFT cycles = cap*512cy; cap=2176 -> 464us @2.4GHz.
"""

import numpy as np

import concourse.bacc as bacc
import concourse.bass_utils as _bu
import concourse.mybir as mybir
import concourse.tile as tile
from concourse.bass import ds, ts
from concourse.bass_utils import run_bass_kernel_spmd

# NOTE: walrus's ldw-opt (--enable-ldw-opt=true) crashes visitInstLdweights
# on this module's bf16 LDWEIGHTS, and the stationary tile changes on every
# matmul here so its dedupe couldn't help anyway — leave it at the default.

P = 128
D_MODEL = 1024
D_FF = 4096
NUM_EXPERTS = 8
TOP_K = 2
NDT = D_MODEL // P   # 8  d-tiles
NFT = D_FF // P      # 32 f-tiles
CAP_DEFAULT = 2176   # tokens-per-expert capacity (multiple of 128)
SUB_MAX = 512        # matmul moving free dim (PSUM fp32 bank limit)

f32 = mybir.dt.float32
bf16 = mybir.dt.bfloat16

_BUILT = {}


def _subs(cap: int):
    """Split [0, cap) into multiples-of-128 chunks in {256, 384, 512}.

    Measured on HW: matmul moving dims that are multiples of 128 stream at
    N/2.4GHz + ~2.5ns; N=448 paid +16ns/MM.  Chunks >=256 keep the
    per-matmul LDWEIGHTS (~97ns) hidden under the matmul stream.  Ascending
    order so the first chunk's input DMA gates the pipeline least."""
    k = cap // 128
    assert cap % 128 == 0 and k >= 5
    r = k % 4
    if r == 0:
        parts = [512] * (k // 4)
    elif r == 1:
        parts = [512] * ((k - 5) // 4) + [384, 256]
    elif r == 2:
        parts = [512] * ((k - 2) // 4) + [256]
    else:
        parts = [512] * ((k - 3) // 4) + [384]
    parts.sort()
    out = []
    off = 0
    for sz in parts:
        out.append((off, sz))
        off += sz
    assert off == cap
    return out


def _bf16(a: np.ndarray) -> np.ndarray:
    """Round-to-nearest-even fp32 -> bf16 (as ml_dtypes.bfloat16 array)."""
    import ml_dtypes

    u = np.ascontiguousarray(a, dtype=np.float32).view(np.uint32)
    u = (u + np.uint32(0x7FFF) + ((u >> np.uint32(16)) & np.uint32(1))) >> np.uint32(16)
    return u.astype(np.uint16).view(ml_dtypes.bfloat16)


def _build(cap: int, repeats: int = 1):
    """Build the per-core expert-MLP Bass module for a given capacity.

    repeats>1 re-runs the whole pipeline (for slope-based HW timing)."""
    nc = bacc.Bacc(None, target_bir_lowering=False)

    # xt / y are flat [P, NDT*cap] with chunk-contiguous host layouts so
    # every DMA moves one contiguous per-partition segment (efficient
    # descriptors).  xt chunk s holds [NDT, sz_s] at flat offset NDT*off_s;
    # y is dt2-major: y[:, dt2*cap + tok].
    xt = nc.declare_dram_parameter("xt", [P, NDT * cap], bf16, isOutput=False)
    w1 = nc.declare_dram_parameter("w1", [P, NFT, NDT, P], bf16, isOutput=False)
    w2 = nc.declare_dram_parameter("w2", [P, NDT, NFT, P], bf16, isOutput=False)
    b1 = nc.declare_dram_parameter("b1", [P, NFT], f32, isOutput=False)
    b2 = nc.declare_dram_parameter("b2", [P, NDT], f32, isOutput=False)
    y = nc.declare_dram_parameter("y", [P, NDT * cap], bf16, isOutput=True)

    subs = _subs(cap)

    with tile.TileContext(nc) as tc:
        with (
            tc.tile_pool(name="const", bufs=1) as const_pool,
            tc.tile_pool(name="xt", bufs=1) as xt_pool,
            tc.tile_pool(name="h", bufs=1) as h_pool,
            tc.tile_pool(name="w1", bufs=2) as w1_pool,
            tc.tile_pool(name="w2", bufs=2) as w2_pool,
            tc.tile_pool(name="yo", bufs=1) as y_pool,
            tc.tile_pool(name="ps1", bufs=4, space="PSUM") as ps1_pool,
            tc.tile_pool(name="ps2", bufs=4, space="PSUM") as ps2_pool,
        ):
            b1_sb = const_pool.tile([P, NFT], f32, name="b1sb")
            b2_sb = const_pool.tile([P, NDT], f32, name="b2sb")
            nc.sync.dma_start(out=b1_sb[:], in_=b1[:])
            nc.sync.dma_start(out=b2_sb[:], in_=b2[:])

            for _ in range(repeats):
                # xt/y ride the Scalar-engine DMA ring; the weight stream
                # (w1t/w2t) keeps the Sync ring to itself, so the 4.5MB
                # input bulk never queues ahead of a 0.26MB weight tile.
                xt_sb = xt_pool.tile([P, NDT * cap], bf16, name="xts")
                for off, sz in subs:
                    nc.scalar.dma_start(
                        out=xt_sb[:, ds(NDT * off, NDT * sz)],
                        in_=xt[:, ds(NDT * off, NDT * sz)],
                    )

                h_sb = h_pool.tile([P, NFT, cap], bf16, name="hsb")

                # ---- Phase 1: h = gelu(X @ W1 + b1), W1 streamed once ----
                for ft in range(NFT):
                    w1t = w1_pool.tile([P, NDT, P], bf16, name="w1t")
                    nc.sync.dma_start(out=w1t[:], in_=w1[:, ts(ft, 1)])
                    for off, sz in subs:
                        ps1 = ps1_pool.tile([P, sz], f32, name="ps1")
                        for dt in range(NDT):
                            nc.tensor.matmul(
                                ps1[:],
                                w1t[:, ts(dt, 1)].squeeze(),
                                xt_sb[:, ds(NDT * off + dt * sz, sz)],
                                start=(dt == 0),
                                stop=(dt == NDT - 1),
                            )
                        nc.scalar.activation(
                            h_sb[:, ts(ft, 1), ds(off, sz)].squeeze(),
                            ps1[:],
                            mybir.ActivationFunctionType.Gelu,
                            bias=b1_sb[:, ts(ft, 1)],
                        )

                # ---- Phase 2: y = h @ W2 + b2, W2 streamed once ----
                for dt2 in range(NDT):
                    w2t = w2_pool.tile([P, NFT, P], bf16, name="w2t")
                    nc.sync.dma_start(out=w2t[:], in_=w2[:, ts(dt2, 1)])
                    y_sb = y_pool.tile([P, cap], bf16, name="ysb")
                    for off, sz in subs:
                        ps2 = ps2_pool.tile([P, sz], f32, name="ps2")
                        for ft in range(NFT):
                            nc.tensor.matmul(
                                ps2[:],
                                w2t[:, ts(ft, 1)].squeeze(),
                                h_sb[:, ts(ft, 1), ds(off, sz)].squeeze(),
                                start=(ft == 0),
                                stop=(ft == NFT - 1),
                            )
                        nc.vector.tensor_scalar_add(
                            y_sb[:, ds(off, sz)],
                            ps2[:],
                            b2_sb[:, ts(dt2, 1)],
                        )
                        nc.scalar.dma_start(
                            out=y[:, ds(dt2 * cap + off, sz)],
                            in_=y_sb[:, ds(off, sz)],
                        )

    nc.compile()
    return nc


def _get_built(cap: int, repeats: int = 1):
    key = (cap, repeats)
    if key not in _BUILT:
        _BUILT[key] = _build(cap, repeats)
    return _BUILT[key]


def _route(x_flat, Wr, br):
    """Router: softmax over experts, top-2, renormalized. Pure numpy."""
    logits = x_flat.astype(np.float32) @ Wr.astype(np.float32) + br.astype(np.float32)
    m = logits.max(axis=-1, keepdims=True)
    p = np.exp(logits - m)
    p /= p.sum(axis=-1, keepdims=True)
    i0 = np.argmax(p, axis=-1)
    pm = p.copy()
    pm[np.arange(p.shape[0]), i0] = -np.inf
    i1 = np.argmax(pm, axis=-1)
    w0 = p[np.arange(p.shape[0]), i0]
    w1 = p[np.arange(p.shape[0]), i1]
    s = w0 + w1
    return i0, i1, w0 / s, w1 / s


def kernel(x, Wr, br, W1, b1, W2, b2, _run_kwargs=None):
    x = np.asarray(x)
    B, L, D = x.shape
    T = B * L
    x_flat = np.ascontiguousarray(x.reshape(T, D), dtype=np.float32)

    i0, i1, w0, w1c = _route(x_flat, Wr, br)

    rows_l, wts_l = [], []
    for e in range(NUM_EXPERTS):
        sel = (i0 == e) | (i1 == e)
        rows = np.nonzero(sel)[0]
        w = np.where(i0[rows] == e, w0[rows], w1c[rows]).astype(np.float32)
        rows_l.append(rows)
        wts_l.append(w)

    max_n = max(len(r) for r in rows_l)
    cap = CAP_DEFAULT
    while cap < max_n:
        cap += 128
    nc = _get_built(cap)

    subs = _subs(cap)
    in_maps = []
    for e in range(NUM_EXPERTS):
        rows = rows_l[e]
        xe = np.zeros((cap, D_MODEL), dtype=np.float32)
        xe[: len(rows)] = x_flat[rows]
        # [cap, D] -> per sub-chunk [P, NDT, sz], flat-concatenated to
        # [P, NDT*cap] so each chunk is one contiguous per-partition block.
        xeT = xe.T.reshape(NDT, P, cap)  # [NDT, P, cap]
        xtr = _bf16(
            np.concatenate(
                [
                    np.ascontiguousarray(
                        xeT[:, :, off : off + sz].transpose(1, 0, 2)
                    ).reshape(P, NDT * sz)
                    for off, sz in subs
                ],
                axis=1,
            )
        )
        w1r = _bf16(
            np.ascontiguousarray(
                np.asarray(W1[e], dtype=np.float32)
                .reshape(NDT, P, NFT, P)
                .transpose(1, 2, 0, 3)
            )
        )
        w2r = _bf16(
            np.ascontiguousarray(
                np.asarray(W2[e], dtype=np.float32)
                .reshape(NFT, P, NDT, P)
                .transpose(1, 2, 0, 3)
            )
        )
        b1r = np.ascontiguousarray(
            np.asarray(b1[e], dtype=np.float32).reshape(NFT, P).T
        )
        b2r = np.ascontiguousarray(
            np.asarray(b2[e], dtype=np.float32).reshape(NDT, P).T
        )
        in_maps.append(
            {"xt": xtr, "w1": w1r, "w2": w2r, "b1": b1r, "b2": b2r}
        )

    kw = dict(_run_kwargs or {})
    res = run_bass_kernel_spmd(nc, in_maps, list(range(NUM_EXPERTS)), **kw)

    out = np.zeros((T, D_MODEL), dtype=np.float32)
    for e in range(NUM_EXPERTS):
        rows = rows_l[e]
        ye = np.asarray(res.results[e]["y"]).astype(np.float32)  # [P, NDT*cap]
        ye = ye.reshape(P, NDT, cap).transpose(1, 0, 2).reshape(D_MODEL, cap)
        out[rows] += wts_l[e][:, None] * ye[:, : len(rows)].T

    kernel._last_result = res
    kernel._last_in_maps = in_maps
    kernel._last_cap = cap
    return out.reshape(B, L, D_MODEL)


def make_bench_runner(nc, in_maps, n_cores=NUM_EXPERTS):
    """Device-resident repeat-execution runner for timing (mirrors
    bass2jax.run_bass_via_pjrt's multi-core path, but stages inputs on
    device once and creates donated zero outputs on-device)."""
    import jax
    import jax.numpy as jnp
    from jax.experimental.shard_map import shard_map
    from jax.sharding import Mesh, NamedSharding, PartitionSpec

    from concourse import bass2jax
    from concourse import mybir as _mybir

    bass2jax.install_neuronx_cc_hook()

    part_name = (
        nc.partition_id_tensor.name if nc.partition_id_tensor else None
    )
    in_names, out_names, out_avals = [], [], []
    for alloc in nc.m.functions[0].allocations:
        if not isinstance(alloc, _mybir.MemoryLocationSet):
            continue
        name = alloc.memorylocations[0].name
        if alloc.kind == "ExternalInput":
            if name != part_name:
                in_names.append(name)
        elif alloc.kind == "ExternalOutput":
            out_names.append(name)
            out_avals.append(
                jax.core.ShapedArray(
                    tuple(alloc.tensor_shape), _mybir.dt.np(alloc.dtype)
                )
            )
    n_params = len(in_names)
    all_in = in_names + out_names
    if part_name is not None:
        all_in = all_in + [part_name]

    def _body(*args):
        operands = list(args)
        if part_name is not None:
            operands.append(bass2jax.partition_id_tensor())
        outs = bass2jax._bass_exec_p.bind(
            *operands,
            out_avals=tuple(out_avals),
            in_names=tuple(all_in),
            out_names=tuple(out_names),
            lowering_input_output_aliases=(),
            sim_require_finite=True,
            sim_require_nnan=True,
            nc=nc,
        )
        return tuple(outs)

    devices = jax.devices()[:n_cores]
    mesh = Mesh(np.asarray(devices), ("core",))
    spec = NamedSharding(mesh, PartitionSpec("core"))
    donate = tuple(range(n_params, n_params + len(out_names)))
    sharded = jax.jit(
        shard_map(
            _body,
            mesh=mesh,
            in_specs=(PartitionSpec("core"),) * (n_params + len(out_names)),
            out_specs=(PartitionSpec("core"),) * len(out_names),
            check_rep=False,
        ),
        donate_argnums=donate,
        keep_unused=True,
    )
    din = [
        jax.device_put(
            np.concatenate([m[name] for m in in_maps], axis=0), spec
        )
        for name in in_names
    ]
    zero_shapes = [
        (n_cores * a.shape[0], *a.shape[1:]) for a in out_avals
    ]
    zeros_fn = jax.jit(
        lambda: tuple(
            jnp.zeros(s, a.dtype) for s, a in zip(zero_shapes, out_avals)
        ),
        out_shardings=tuple(spec for _ in out_avals),
    )

    def run_once():
        return sharded(*din, *zeros_fn())

    def zeros_only():
        return zeros_fn()

    return run_once, zeros_only


# revision 12
# speedup vs baseline: 1.0075x; 1.0075x over previous
"""MoE (8 experts, top-2) Trainium2 kernel.

Strategy (expert-parallel, per sharding hint):
  - Host: router (softmax + top-2 + renorm)  [0.1% of FLOPs], then
    all-to-all dispatch done host-side: gather each expert's tokens into a
    fixed-capacity buffer (bf16).
  - Device (8 cores, 1 expert each): Y_e = GELU(X_e @ W1[e] + b1[e]) @ W2[e] + b2[e]
    All matmuls bf16 (full 1-row/cycle PE rate).  Weights are streamed
    from HBM exactly ONCE (vs once per token-chunk): X (bf16, 4.5MB) and
    the full hidden activation h (bf16, 17.8MB) stay resident in SBUF.
  - Host: weighted combine (the return all-to-all) back to [B, L, D].

Device schedule per core (capacity `cap` tokens, sub-chunks of <=512):
  Phase 1 (per ft in 32):  dma W1[:,ft];  per sub:
      ps1[f128, sub] = sum_dt W1t[dt].T @ X[dt, sub]      (8 mms, bf16)
      h[ft, sub] = Gelu(ps1 + b1[ft])                     (ScalarE -> bf16)
  Phase 2 (per dt2 in 8):  dma W2[:,dt2];  per sub:
      ps2[d128, sub] = sum_ft W2t[ft].T @ h[ft, sub]      (32 mms, bf16)
      y[dt2, sub] = ps2 + b2[dt2]                         (DVE -> bf16)

PSUM: ps1 pool 4 banks + ps2 pool 4 banks = 8.  PE never waits on PSUM.
Compute floor: 2*cap*NDT*NFT cycles = cap*512cy; cap=2176 -> 464us @2.4GHz.
"""

import numpy as np

import concourse.bacc as bacc
import concourse.bass_utils as _bu
import concourse.mybir as mybir
import concourse.tile as tile
from concourse.bass import ds, ts
from concourse.bass_utils import run_bass_kernel_spmd

# NOTE: walrus's ldw-opt (--enable-ldw-opt=true) crashes visitInstLdweights
# on this module's bf16 LDWEIGHTS, and the stationary tile changes on every
# matmul here so its dedupe couldn't help anyway — leave it at the default.

P = 128
D_MODEL = 1024
D_FF = 4096
NUM_EXPERTS = 8
TOP_K = 2
NDT = D_MODEL // P   # 8  d-tiles
NFT = D_FF // P      # 32 f-tiles
CAP_DEFAULT = 2176   # tokens-per-expert capacity (multiple of 128)
SUB_MAX = 512        # matmul moving free dim (PSUM fp32 bank limit)

f32 = mybir.dt.float32
bf16 = mybir.dt.bfloat16

_BUILT = {}


def _subs(cap: int):
    """Split [0, cap) into multiples-of-128 chunks in {256, 384, 512}.

    Measured on HW: matmul moving dims that are multiples of 128 stream at
    N/2.4GHz + ~2.5ns; N=448 paid +16ns/MM.  Chunks >=256 keep the
    per-matmul LDWEIGHTS (~97ns) hidden under the matmul stream.  Ascending
    order so the first chunk's input DMA gates the pipeline least."""
    k = cap // 128
    assert cap % 128 == 0 and k >= 5
    r = k % 3
    if r == 0:
        parts = [384] * (k // 3)
    elif r == 1:
        parts = [384] * ((k - 4) // 3) + [256, 256]
    else:
        parts = [384] * ((k - 2) // 3) + [256]
    parts.sort()
    out = []
    off = 0
    for sz in parts:
        out.append((off, sz))
        off += sz
    assert off == cap
    return out


def _bf16(a: np.ndarray) -> np.ndarray:
    """Round-to-nearest-even fp32 -> bf16 (as ml_dtypes.bfloat16 array)."""
    import ml_dtypes

    u = np.ascontiguousarray(a, dtype=np.float32).view(np.uint32)
    u = (u + np.uint32(0x7FFF) + ((u >> np.uint32(16)) & np.uint32(1))) >> np.uint32(16)
    return u.astype(np.uint16).view(ml_dtypes.bfloat16)


def _build(cap: int, repeats: int = 1):
    """Build the per-core expert-MLP Bass module for a given capacity.

    repeats>1 re-runs the whole pipeline (for slope-based HW timing)."""
    nc = bacc.Bacc(None, target_bir_lowering=False)

    # xt / y are flat [P, NDT*cap] with chunk-contiguous host layouts so
    # every DMA moves one contiguous per-partition segment (efficient
    # descriptors).  xt chunk s holds [NDT, sz_s] at flat offset NDT*off_s;
    # y is dt2-major: y[:, dt2*cap + tok].
    xt = nc.declare_dram_parameter("xt", [P, NDT * cap], bf16, isOutput=False)
    w1 = nc.declare_dram_parameter("w1", [P, NFT, NDT, P], bf16, isOutput=False)
    w2 = nc.declare_dram_parameter("w2", [P, NDT, NFT, P], bf16, isOutput=False)
    b1 = nc.declare_dram_parameter("b1", [P, NFT], f32, isOutput=False)
    b2 = nc.declare_dram_parameter("b2", [P, NDT], f32, isOutput=False)
    y = nc.declare_dram_parameter("y", [P, NDT * cap], bf16, isOutput=True)

    subs = _subs(cap)

    with tile.TileContext(nc) as tc:
        with (
            tc.tile_pool(name="const", bufs=1) as const_pool,
            tc.tile_pool(name="xt", bufs=1) as xt_pool,
            tc.tile_pool(name="h", bufs=1) as h_pool,
            tc.tile_pool(name="w1", bufs=2) as w1_pool,
            tc.tile_pool(name="w2", bufs=2) as w2_pool,
            tc.tile_pool(name="yo", bufs=1) as y_pool,
            tc.tile_pool(name="ps1", bufs=4, space="PSUM") as ps1_pool,
            tc.tile_pool(name="ps2", bufs=4, space="PSUM") as ps2_pool,
        ):
            b1_sb = const_pool.tile([P, NFT], f32, name="b1sb")
            b2_sb = const_pool.tile([P, NDT], f32, name="b2sb")
            nc.sync.dma_start(out=b1_sb[:], in_=b1[:])
            nc.sync.dma_start(out=b2_sb[:], in_=b2[:])

            for _ in range(repeats):
                # xt/y ride the Scalar-engine DMA ring; the weight stream
                # (w1t/w2t) keeps the Sync ring to itself, so the 4.5MB
                # input bulk never queues ahead of a 0.26MB weight tile.
                xt_sb = xt_pool.tile([P, NDT * cap], bf16, name="xts")
                for off, sz in subs:
                    nc.scalar.dma_start(
                        out=xt_sb[:, ds(NDT * off, NDT * sz)],
                        in_=xt[:, ds(NDT * off, NDT * sz)],
                    )

                h_sb = h_pool.tile([P, NFT, cap], bf16, name="hsb")

                # ---- Phase 1: h = gelu(X @ W1 + b1), W1 streamed once ----
                for ft in range(NFT):
                    w1t = w1_pool.tile([P, NDT, P], bf16, name="w1t")
                    nc.sync.dma_start(out=w1t[:], in_=w1[:, ts(ft, 1)])
                    for off, sz in subs:
                        ps1 = ps1_pool.tile([P, sz], f32, name="ps1")
                        for dt in range(NDT):
                            nc.tensor.matmul(
                                ps1[:],
                                w1t[:, ts(dt, 1)].squeeze(),
                                xt_sb[:, ds(NDT * off + dt * sz, sz)],
                                start=(dt == 0),
                                stop=(dt == NDT - 1),
                            )
                        nc.scalar.activation(
                            h_sb[:, ts(ft, 1), ds(off, sz)].squeeze(),
                            ps1[:],
                            mybir.ActivationFunctionType.Gelu,
                            bias=b1_sb[:, ts(ft, 1)],
                        )

                # ---- Phase 2: y = h @ W2 + b2, W2 streamed once ----
                for dt2 in range(NDT):
                    w2t = w2_pool.tile([P, NFT, P], bf16, name="w2t")
                    nc.sync.dma_start(out=w2t[:], in_=w2[:, ts(dt2, 1)])
                    y_sb = y_pool.tile([P, cap], bf16, name="ysb")
                    for off, sz in subs:
                        ps2 = ps2_pool.tile([P, sz], f32, name="ps2")
                        for ft in range(NFT):
                            nc.tensor.matmul(
                                ps2[:],
                                w2t[:, ts(ft, 1)].squeeze(),
                                h_sb[:, ts(ft, 1), ds(off, sz)].squeeze(),
                                start=(ft == 0),
                                stop=(ft == NFT - 1),
                            )
                        nc.vector.tensor_scalar_add(
                            y_sb[:, ds(off, sz)],
                            ps2[:],
                            b2_sb[:, ts(dt2, 1)],
                        )
                        nc.scalar.dma_start(
                            out=y[:, ds(dt2 * cap + off, sz)],
                            in_=y_sb[:, ds(off, sz)],
                        )

    nc.compile()
    return nc


def _get_built(cap: int, repeats: int = 1):
    key = (cap, repeats)
    if key not in _BUILT:
        _BUILT[key] = _build(cap, repeats)
    return _BUILT[key]


def _route(x_flat, Wr, br):
    """Router: softmax over experts, top-2, renormalized. Pure numpy."""
    logits = x_flat.astype(np.float32) @ Wr.astype(np.float32) + br.astype(np.float32)
    m = logits.max(axis=-1, keepdims=True)
    p = np.exp(logits - m)
    p /= p.sum(axis=-1, keepdims=True)
    i0 = np.argmax(p, axis=-1)
    pm = p.copy()
    pm[np.arange(p.shape[0]), i0] = -np.inf
    i1 = np.argmax(pm, axis=-1)
    w0 = p[np.arange(p.shape[0]), i0]
    w1 = p[np.arange(p.shape[0]), i1]
    s = w0 + w1
    return i0, i1, w0 / s, w1 / s


def kernel(x, Wr, br, W1, b1, W2, b2, _run_kwargs=None):
    x = np.asarray(x)
    B, L, D = x.shape
    T = B * L
    x_flat = np.ascontiguousarray(x.reshape(T, D), dtype=np.float32)

    i0, i1, w0, w1c = _route(x_flat, Wr, br)

    rows_l, wts_l = [], []
    for e in range(NUM_EXPERTS):
        sel = (i0 == e) | (i1 == e)
        rows = np.nonzero(sel)[0]
        w = np.where(i0[rows] == e, w0[rows], w1c[rows]).astype(np.float32)
        rows_l.append(rows)
        wts_l.append(w)

    max_n = max(len(r) for r in rows_l)
    cap = CAP_DEFAULT
    while cap < max_n:
        cap += 128
    nc = _get_built(cap)

    subs = _subs(cap)
    in_maps = []
    for e in range(NUM_EXPERTS):
        rows = rows_l[e]
        xe = np.zeros((cap, D_MODEL), dtype=np.float32)
        xe[: len(rows)] = x_flat[rows]
        # [cap, D] -> per sub-chunk [P, NDT, sz], flat-concatenated to
        # [P, NDT*cap] so each chunk is one contiguous per-partition block.
        xeT = xe.T.reshape(NDT, P, cap)  # [NDT, P, cap]
        xtr = _bf16(
            np.concatenate(
                [
                    np.ascontiguousarray(
                        xeT[:, :, off : off + sz].transpose(1, 0, 2)
                    ).reshape(P, NDT * sz)
                    for off, sz in subs
                ],
                axis=1,
            )
        )
        w1r = _bf16(
            np.ascontiguousarray(
                np.asarray(W1[e], dtype=np.float32)
                .reshape(NDT, P, NFT, P)
                .transpose(1, 2, 0, 3)
            )
        )
        w2r = _bf16(
            np.ascontiguousarray(
                np.asarray(W2[e], dtype=np.float32)
                .reshape(NFT, P, NDT, P)
                .transpose(1, 2, 0, 3)
            )
        )
        b1r = np.ascontiguousarray(
            np.asarray(b1[e], dtype=np.float32).reshape(NFT, P).T
        )
        b2r = np.ascontiguousarray(
            np.asarray(b2[e], dtype=np.float32).reshape(NDT, P).T
        )
        in_maps.append(
            {"xt": xtr, "w1": w1r, "w2": w2r, "b1": b1r, "b2": b2r}
        )

    kw = dict(_run_kwargs or {})
    res = run_bass_kernel_spmd(nc, in_maps, list(range(NUM_EXPERTS)), **kw)

    out = np.zeros((T, D_MODEL), dtype=np.float32)
    for e in range(NUM_EXPERTS):
        rows = rows_l[e]
        ye = np.asarray(res.results[e]["y"]).astype(np.float32)  # [P, NDT*cap]
        ye = ye.reshape(P, NDT, cap).transpose(1, 0, 2).reshape(D_MODEL, cap)
        out[rows] += wts_l[e][:, None] * ye[:, : len(rows)].T

    kernel._last_result = res
    kernel._last_in_maps = in_maps
    kernel._last_cap = cap
    return out.reshape(B, L, D_MODEL)


def make_bench_runner(nc, in_maps, n_cores=NUM_EXPERTS):
    """Device-resident repeat-execution runner for timing (mirrors
    bass2jax.run_bass_via_pjrt's multi-core path, but stages inputs on
    device once and creates donated zero outputs on-device)."""
    import jax
    import jax.numpy as jnp
    from jax.experimental.shard_map import shard_map
    from jax.sharding import Mesh, NamedSharding, PartitionSpec

    from concourse import bass2jax
    from concourse import mybir as _mybir

    bass2jax.install_neuronx_cc_hook()

    part_name = (
        nc.partition_id_tensor.name if nc.partition_id_tensor else None
    )
    in_names, out_names, out_avals = [], [], []
    for alloc in nc.m.functions[0].allocations:
        if not isinstance(alloc, _mybir.MemoryLocationSet):
            continue
        name = alloc.memorylocations[0].name
        if alloc.kind == "ExternalInput":
            if name != part_name:
                in_names.append(name)
        elif alloc.kind == "ExternalOutput":
            out_names.append(name)
            out_avals.append(
                jax.core.ShapedArray(
                    tuple(alloc.tensor_shape), _mybir.dt.np(alloc.dtype)
                )
            )
    n_params = len(in_names)
    all_in = in_names + out_names
    if part_name is not None:
        all_in = all_in + [part_name]

    def _body(*args):
        operands = list(args)
        if part_name is not None:
            operands.append(bass2jax.partition_id_tensor())
        outs = bass2jax._bass_exec_p.bind(
            *operands,
            out_avals=tuple(out_avals),
            in_names=tuple(all_in),
            out_names=tuple(out_names),
            lowering_input_output_aliases=(),
            sim_require_finite=True,
            sim_require_nnan=True,
            nc=nc,
        )
        return tuple(outs)

    devices = jax.devices()[:n_cores]
    mesh = Mesh(np.asarray(devices), ("core",))
    spec = NamedSharding(mesh, PartitionSpec("core"))
    donate = tuple(range(n_params, n_params + len(out_names)))
    sharded = jax.jit(
        shard_map(
            _body,
            mesh=mesh,
            in_specs=(PartitionSpec("core"),) * (n_params + len(out_names)),
            out_specs=(PartitionSpec("core"),) * len(out_names),
            check_rep=False,
        ),
        donate_argnums=donate,
        keep_unused=True,
    )
    din = [
        jax.device_put(
            np.concatenate([m[name] for m in in_maps], axis=0), spec
        )
        for name in in_names
    ]
    zero_shapes = [
        (n_cores * a.shape[0], *a.shape[1:]) for a in out_avals
    ]
    zeros_fn = jax.jit(
        lambda: tuple(
            jnp.zeros(s, a.dtype) for s, a in zip(zero_shapes, out_avals)
        ),
        out_shardings=tuple(spec for _ in out_avals),
    )

    def run_once():
        return sharded(*din, *zeros_fn())

    def zeros_only():
        return zeros_fn()

    return run_once, zeros_only


# revision 15
# speedup vs baseline: 1.0736x; 1.0656x over previous
"""MoE (8 experts, top-2) Trainium2 kernel.

Strategy (expert-parallel, per sharding hint):
  - Host: router (softmax + top-2 + renorm)  [0.1% of FLOPs], then
    all-to-all dispatch done host-side: gather each expert's tokens into a
    fixed-capacity buffer (bf16).
  - Device (8 cores, 1 expert each): Y_e = GELU(X_e @ W1[e] + b1[e]) @ W2[e] + b2[e]
    All matmuls bf16 (full 1-row/cycle PE rate).  Weights are streamed
    from HBM exactly ONCE (vs once per token-chunk): X (bf16, 4.5MB) and
    the full hidden activation h (bf16, 17.8MB) stay resident in SBUF.
  - Host: weighted combine (the return all-to-all) back to [B, L, D].

Device schedule per core (capacity `cap` tokens, sub-chunks of <=512):
  Phase 1 (per ft in 32):  dma W1[:,ft];  per sub:
      ps1[f128, sub] = sum_dt W1t[dt].T @ X[dt, sub]      (8 mms, bf16)
      h[ft, sub] = Gelu(ps1 + b1[ft])                     (ScalarE -> bf16)
  Phase 2 (per dt2 in 8):  dma W2[:,dt2];  per sub:
      ps2[d128, sub] = sum_ft W2t[ft].T @ h[ft, sub]      (32 mms, bf16)
      y[dt2, sub] = ps2 + b2[dt2]                         (DVE -> bf16)

PSUM: ps1 pool 4 banks + ps2 pool 4 banks = 8.  PE never waits on PSUM.
Compute floor: 2*cap*NDT*NFT cycles = cap*512cy; cap=2176 -> 464us @2.4GHz.
"""

import numpy as np

import concourse.bacc as bacc
import concourse.bass_utils as _bu
import concourse.mybir as mybir
import concourse.tile as tile
from concourse.bass import ds, ts
from concourse.bass_utils import run_bass_kernel_spmd

# NOTE: walrus's ldw-opt (--enable-ldw-opt=true) crashes visitInstLdweights
# on this module's bf16 LDWEIGHTS, and the stationary tile changes on every
# matmul here so its dedupe couldn't help anyway — leave it at the default.

P = 128
D_MODEL = 1024
D_FF = 4096
NUM_EXPERTS = 8
TOP_K = 2
NDT = D_MODEL // P   # 8  d-tiles
NFT = D_FF // P      # 32 f-tiles
CAP_DEFAULT = 2176   # tokens-per-expert capacity (multiple of 128)
SUB_MAX = 512        # matmul moving free dim (PSUM fp32 bank limit)

f32 = mybir.dt.float32
bf16 = mybir.dt.bfloat16

_BUILT = {}


def _subs(cap: int):
    """Split [0, cap) into multiples-of-128 chunks in {256, 384, 512}.

    Measured on HW: matmul moving dims that are multiples of 128 stream at
    N/2.4GHz + ~2.5ns; N=448 paid +16ns/MM.  Chunks >=256 keep the
    per-matmul LDWEIGHTS (~97ns) hidden under the matmul stream.  Ascending
    order so the first chunk's input DMA gates the pipeline least."""
    k = cap // 128
    assert cap % 128 == 0 and k >= 5
    r = k % 4
    if r == 0:
        parts = [512] * (k // 4)
    elif r == 1:
        parts = [512] * ((k - 5) // 4) + [384, 256]
    elif r == 2:
        parts = [512] * ((k - 2) // 4) + [256]
    else:
        parts = [512] * ((k - 3) // 4) + [384]
    parts.sort()
    out = []
    off = 0
    for sz in parts:
        out.append((off, sz))
        off += sz
    assert off == cap
    return out


def _bf16(a: np.ndarray) -> np.ndarray:
    """Round-to-nearest-even fp32 -> bf16 (as ml_dtypes.bfloat16 array)."""
    import ml_dtypes

    u = np.ascontiguousarray(a, dtype=np.float32).view(np.uint32)
    u = (u + np.uint32(0x7FFF) + ((u >> np.uint32(16)) & np.uint32(1))) >> np.uint32(16)
    return u.astype(np.uint16).view(ml_dtypes.bfloat16)


def _build(cap: int, repeats: int = 1):
    """Build the per-core expert-MLP Bass module for a given capacity.

    repeats>1 re-runs the whole pipeline (for slope-based HW timing)."""
    nc = bacc.Bacc(None, target_bir_lowering=False)

    # xt / y are flat [P, NDT*cap] with chunk-contiguous host layouts so
    # every DMA moves one contiguous per-partition segment (efficient
    # descriptors).  xt chunk s holds [NDT, sz_s] at flat offset NDT*off_s;
    # y is dt2-major: y[:, dt2*cap + tok].
    xt = nc.declare_dram_parameter("xt", [P, NDT * cap], bf16, isOutput=False)
    w1 = nc.declare_dram_parameter("w1", [P, NFT, NDT, P], bf16, isOutput=False)
    w2 = nc.declare_dram_parameter("w2", [P, NDT, NFT, P], bf16, isOutput=False)
    b1 = nc.declare_dram_parameter("b1", [P, NFT], f32, isOutput=False)
    b2 = nc.declare_dram_parameter("b2", [P, NDT], f32, isOutput=False)
    y = nc.declare_dram_parameter("y", [P, NDT * cap], bf16, isOutput=True)

    subs = _subs(cap)

    with tile.TileContext(nc) as tc:
        with (
            tc.tile_pool(name="const", bufs=1) as const_pool,
            tc.tile_pool(name="xt", bufs=1) as xt_pool,
            tc.tile_pool(name="h", bufs=1) as h_pool,
            tc.tile_pool(name="w1", bufs=3) as w1_pool,
            tc.tile_pool(name="w2", bufs=2) as w2_pool,
            tc.tile_pool(name="yo", bufs=1) as y_pool,
            tc.tile_pool(name="ps1", bufs=4, space="PSUM") as ps1_pool,
            tc.tile_pool(name="ps2", bufs=4, space="PSUM") as ps2_pool,
        ):
            b1_sb = const_pool.tile([P, NFT], f32, name="b1sb")
            b2_sb = const_pool.tile([P, NDT], f32, name="b2sb")

            for rep in range(repeats):
                # DMA startup order matters: the first weight tile must not
                # queue behind the 4.5MB xt bulk.  Sync ring carries the
                # weight stream (w1t0 first) plus every other xt chunk
                # interleaved after the early weight tiles; the Scalar ring
                # carries the remaining xt chunks (and, in phase 2, y-out).
                xt_sb = xt_pool.tile([P, NDT * cap], bf16, name="xts")
                h_sb = h_pool.tile([P, NFT, cap], bf16, name="hsb")
                w1_tiles = {}
                w1_tiles[0] = w1_pool.tile([P, NDT, P], bf16, name="w1t")
                nc.sync.dma_start(out=w1_tiles[0][:], in_=w1[:, ts(0, 1)])
                if rep == 0:
                    nc.sync.dma_start(out=b1_sb[:], in_=b1[:])
                    nc.sync.dma_start(out=b2_sb[:], in_=b2[:])
                for i, (off, sz) in enumerate(subs):
                    eng = nc.scalar if i % 2 == 0 else nc.sync
                    if i % 2 == 1:
                        # prefetch the next weight tile between sync-ring
                        # xt chunks so the weight stream stays ahead
                        ftn = (i + 1) // 2
                        if ftn < NFT and ftn not in w1_tiles:
                            w1_tiles[ftn] = w1_pool.tile(
                                [P, NDT, P], bf16, name="w1t"
                            )
                            nc.sync.dma_start(
                                out=w1_tiles[ftn][:], in_=w1[:, ts(ftn, 1)]
                            )
                    eng.dma_start(
                        out=xt_sb[:, ds(NDT * off, NDT * sz)],
                        in_=xt[:, ds(NDT * off, NDT * sz)],
                    )

                # ---- Phase 1: h = gelu(X @ W1 + b1), W1 streamed once ----
                for ft in range(NFT):
                    if ft in w1_tiles:
                        w1t = w1_tiles[ft]
                    else:
                        w1t = w1_pool.tile([P, NDT, P], bf16, name="w1t")
                        nc.sync.dma_start(out=w1t[:], in_=w1[:, ts(ft, 1)])
                    for off, sz in subs:
                        ps1 = ps1_pool.tile([P, sz], f32, name="ps1")
                        for dt in range(NDT):
                            nc.tensor.matmul(
                                ps1[:],
                                w1t[:, ts(dt, 1)].squeeze(),
                                xt_sb[:, ds(NDT * off + dt * sz, sz)],
                                start=(dt == 0),
                                stop=(dt == NDT - 1),
                            )
                        nc.scalar.activation(
                            h_sb[:, ts(ft, 1), ds(off, sz)].squeeze(),
                            ps1[:],
                            mybir.ActivationFunctionType.Gelu,
                            bias=b1_sb[:, ts(ft, 1)],
                        )

                # ---- Phase 2: y = h @ W2 + b2, W2 streamed once ----
                for dt2 in range(NDT):
                    w2t = w2_pool.tile([P, NFT, P], bf16, name="w2t")
                    nc.sync.dma_start(out=w2t[:], in_=w2[:, ts(dt2, 1)])
                    y_sb = y_pool.tile([P, cap], bf16, name="ysb")
                    for off, sz in subs:
                        ps2 = ps2_pool.tile([P, sz], f32, name="ps2")
                        for ft in range(NFT):
                            nc.tensor.matmul(
                                ps2[:],
                                w2t[:, ts(ft, 1)].squeeze(),
                                h_sb[:, ts(ft, 1), ds(off, sz)].squeeze(),
                                start=(ft == 0),
                                stop=(ft == NFT - 1),
                            )
                        nc.vector.tensor_scalar_add(
                            y_sb[:, ds(off, sz)],
                            ps2[:],
                            b2_sb[:, ts(dt2, 1)],
                        )
                        nc.scalar.dma_start(
                            out=y[:, ds(dt2 * cap + off, sz)],
                            in_=y_sb[:, ds(off, sz)],
                        )

    nc.compile()
    return nc


def _get_built(cap: int, repeats: int = 1):
    key = (cap, repeats)
    if key not in _BUILT:
        _BUILT[key] = _build(cap, repeats)
    return _BUILT[key]


def _route(x_flat, Wr, br):
    """Router: softmax over experts, top-2, renormalized. Pure numpy."""
    logits = x_flat.astype(np.float32) @ Wr.astype(np.float32) + br.astype(np.float32)
    m = logits.max(axis=-1, keepdims=True)
    p = np.exp(logits - m)
    p /= p.sum(axis=-1, keepdims=True)
    i0 = np.argmax(p, axis=-1)
    pm = p.copy()
    pm[np.arange(p.shape[0]), i0] = -np.inf
    i1 = np.argmax(pm, axis=-1)
    w0 = p[np.arange(p.shape[0]), i0]
    w1 = p[np.arange(p.shape[0]), i1]
    s = w0 + w1
    return i0, i1, w0 / s, w1 / s


def kernel(x, Wr, br, W1, b1, W2, b2, _run_kwargs=None):
    x = np.asarray(x)
    B, L, D = x.shape
    T = B * L
    x_flat = np.ascontiguousarray(x.reshape(T, D), dtype=np.float32)

    i0, i1, w0, w1c = _route(x_flat, Wr, br)

    rows_l, wts_l = [], []
    for e in range(NUM_EXPERTS):
        sel = (i0 == e) | (i1 == e)
        rows = np.nonzero(sel)[0]
        w = np.where(i0[rows] == e, w0[rows], w1c[rows]).astype(np.float32)
        rows_l.append(rows)
        wts_l.append(w)

    max_n = max(len(r) for r in rows_l)
    cap = CAP_DEFAULT
    while cap < max_n:
        cap += 128
    nc = _get_built(cap)

    subs = _subs(cap)
    in_maps = []
    for e in range(NUM_EXPERTS):
        rows = rows_l[e]
        xe = np.zeros((cap, D_MODEL), dtype=np.float32)
        xe[: len(rows)] = x_flat[rows]
        # [cap, D] -> per sub-chunk [P, NDT, sz], flat-concatenated to
        # [P, NDT*cap] so each chunk is one contiguous per-partition block.
        xeT = xe.T.reshape(NDT, P, cap)  # [NDT, P, cap]
        xtr = _bf16(
            np.concatenate(
                [
                    np.ascontiguousarray(
                        xeT[:, :, off : off + sz].transpose(1, 0, 2)
                    ).reshape(P, NDT * sz)
                    for off, sz in subs
                ],
                axis=1,
            )
        )
        w1r = _bf16(
            np.ascontiguousarray(
                np.asarray(W1[e], dtype=np.float32)
                .reshape(NDT, P, NFT, P)
                .transpose(1, 2, 0, 3)
            )
        )
        w2r = _bf16(
            np.ascontiguousarray(
                np.asarray(W2[e], dtype=np.float32)
                .reshape(NFT, P, NDT, P)
                .transpose(1, 2, 0, 3)
            )
        )
        b1r = np.ascontiguousarray(
            np.asarray(b1[e], dtype=np.float32).reshape(NFT, P).T
        )
        b2r = np.ascontiguousarray(
            np.asarray(b2[e], dtype=np.float32).reshape(NDT, P).T
        )
        in_maps.append(
            {"xt": xtr, "w1": w1r, "w2": w2r, "b1": b1r, "b2": b2r}
        )

    kw = dict(_run_kwargs or {})
    res = run_bass_kernel_spmd(nc, in_maps, list(range(NUM_EXPERTS)), **kw)

    out = np.zeros((T, D_MODEL), dtype=np.float32)
    for e in range(NUM_EXPERTS):
        rows = rows_l[e]
        ye = np.asarray(res.results[e]["y"]).astype(np.float32)  # [P, NDT*cap]
        ye = ye.reshape(P, NDT, cap).transpose(1, 0, 2).reshape(D_MODEL, cap)
        out[rows] += wts_l[e][:, None] * ye[:, : len(rows)].T

    kernel._last_result = res
    kernel._last_in_maps = in_maps
    kernel._last_cap = cap
    return out.reshape(B, L, D_MODEL)


def make_bench_runner(nc, in_maps, n_cores=NUM_EXPERTS):
    """Device-resident repeat-execution runner for timing (mirrors
    bass2jax.run_bass_via_pjrt's multi-core path, but stages inputs on
    device once and creates donated zero outputs on-device)."""
    import jax
    import jax.numpy as jnp
    from jax.experimental.shard_map import shard_map
    from jax.sharding import Mesh, NamedSharding, PartitionSpec

    from concourse import bass2jax
    from concourse import mybir as _mybir

    bass2jax.install_neuronx_cc_hook()

    part_name = (
        nc.partition_id_tensor.name if nc.partition_id_tensor else None
    )
    in_names, out_names, out_avals = [], [], []
    for alloc in nc.m.functions[0].allocations:
        if not isinstance(alloc, _mybir.MemoryLocationSet):
            continue
        name = alloc.memorylocations[0].name
        if alloc.kind == "ExternalInput":
            if name != part_name:
                in_names.append(name)
        elif alloc.kind == "ExternalOutput":
            out_names.append(name)
            out_avals.append(
                jax.core.ShapedArray(
                    tuple(alloc.tensor_shape), _mybir.dt.np(alloc.dtype)
                )
            )
    n_params = len(in_names)
    all_in = in_names + out_names
    if part_name is not None:
        all_in = all_in + [part_name]

    def _body(*args):
        operands = list(args)
        if part_name is not None:
            operands.append(bass2jax.partition_id_tensor())
        outs = bass2jax._bass_exec_p.bind(
            *operands,
            out_avals=tuple(out_avals),
            in_names=tuple(all_in),
            out_names=tuple(out_names),
            lowering_input_output_aliases=(),
            sim_require_finite=True,
            sim_require_nnan=True,
            nc=nc,
        )
        return tuple(outs)

    devices = jax.devices()[:n_cores]
    mesh = Mesh(np.asarray(devices), ("core",))
    spec = NamedSharding(mesh, PartitionSpec("core"))
    donate = tuple(range(n_params, n_params + len(out_names)))
    sharded = jax.jit(
        shard_map(
            _body,
            mesh=mesh,
            in_specs=(PartitionSpec("core"),) * (n_params + len(out_names)),
            out_specs=(PartitionSpec("core"),) * len(out_names),
            check_rep=False,
        ),
        donate_argnums=donate,
        keep_unused=True,
    )
    din = [
        jax.device_put(
            np.concatenate([m[name] for m in in_maps], axis=0), spec
        )
        for name in in_names
    ]
    zero_shapes = [
        (n_cores * a.shape[0], *a.shape[1:]) for a in out_avals
    ]
    zeros_fn = jax.jit(
        lambda: tuple(
            jnp.zeros(s, a.dtype) for s, a in zip(zero_shapes, out_avals)
        ),
        out_shardings=tuple(spec for _ in out_avals),
    )

    def run_once():
        return sharded(*din, *zeros_fn())

    def zeros_only():
        return zeros_fn()

    return run_once, zeros_only


# revision 16
# speedup vs baseline: 1.0747x; 1.0010x over previous
"""MoE (8 experts, top-2) Trainium2 kernel.

Strategy (expert-parallel, per sharding hint):
  - Host: router (softmax + top-2 + renorm)  [0.1% of FLOPs], then
    all-to-all dispatch done host-side: gather each expert's tokens into a
    fixed-capacity buffer (bf16).
  - Device (8 cores, 1 expert each): Y_e = GELU(X_e @ W1[e] + b1[e]) @ W2[e] + b2[e]
    All matmuls bf16 (full 1-row/cycle PE rate).  Weights are streamed
    from HBM exactly ONCE (vs once per token-chunk): X (bf16, 4.5MB) and
    the full hidden activation h (bf16, 17.8MB) stay resident in SBUF.
  - Host: weighted combine (the return all-to-all) back to [B, L, D].

Device schedule per core (capacity `cap` tokens, sub-chunks of <=512):
  Phase 1 (per ft in 32):  dma W1[:,ft];  per sub:
      ps1[f128, sub] = sum_dt W1t[dt].T @ X[dt, sub]      (8 mms, bf16)
      h[ft, sub] = Gelu(ps1 + b1[ft])                     (ScalarE -> bf16)
  Phase 2 (per dt2 in 8):  dma W2[:,dt2];  per sub:
      ps2[d128, sub] = sum_ft W2t[ft].T @ h[ft, sub]      (32 mms, bf16)
      y[dt2, sub] = ps2 + b2[dt2]                         (DVE -> bf16)

PSUM: ps1 pool 4 banks + ps2 pool 4 banks = 8.  PE never waits on PSUM.
Compute floor: 2*cap*NDT*NFT cycles = cap*512cy; cap=2176 -> 464us @2.4GHz.
"""

import numpy as np

import concourse.bacc as bacc
import concourse.bass_utils as _bu
import concourse.mybir as mybir
import concourse.tile as tile
from concourse.bass import ds, ts
from concourse.bass_utils import run_bass_kernel_spmd

# NOTE: walrus's ldw-opt (--enable-ldw-opt=true) crashes visitInstLdweights
# on this module's bf16 LDWEIGHTS, and the stationary tile changes on every
# matmul here so its dedupe couldn't help anyway — leave it at the default.

P = 128
D_MODEL = 1024
D_FF = 4096
NUM_EXPERTS = 8
TOP_K = 2
NDT = D_MODEL // P   # 8  d-tiles
NFT = D_FF // P      # 32 f-tiles
CAP_DEFAULT = 2176   # tokens-per-expert capacity (multiple of 128)
SUB_MAX = 512        # matmul moving free dim (PSUM fp32 bank limit)

f32 = mybir.dt.float32
bf16 = mybir.dt.bfloat16

_BUILT = {}


def _subs(cap: int):
    """Split [0, cap) into multiples-of-128 chunks in {256, 384, 512}.

    Measured on HW: matmul moving dims that are multiples of 128 stream at
    N/2.4GHz + ~2.5ns; N=448 paid +16ns/MM.  Chunks >=256 keep the
    per-matmul LDWEIGHTS (~97ns) hidden under the matmul stream.  Ascending
    order so the first chunk's input DMA gates the pipeline least."""
    k = cap // 128
    assert cap % 128 == 0 and k >= 5
    r = k % 4
    if r == 0:
        parts = [512] * (k // 4)
    elif r == 1:
        parts = [512] * ((k - 5) // 4) + [384, 256]
    elif r == 2:
        parts = [512] * ((k - 2) // 4) + [256]
    else:
        parts = [512] * ((k - 3) // 4) + [384]
    parts.sort()
    out = []
    off = 0
    for sz in parts:
        out.append((off, sz))
        off += sz
    assert off == cap
    return out


def _bf16(a: np.ndarray) -> np.ndarray:
    """Round-to-nearest-even fp32 -> bf16 (as ml_dtypes.bfloat16 array)."""
    import ml_dtypes

    u = np.ascontiguousarray(a, dtype=np.float32).view(np.uint32)
    u = (u + np.uint32(0x7FFF) + ((u >> np.uint32(16)) & np.uint32(1))) >> np.uint32(16)
    return u.astype(np.uint16).view(ml_dtypes.bfloat16)


def _build(cap: int, repeats: int = 1):
    """Build the per-core expert-MLP Bass module for a given capacity.

    repeats>1 re-runs the whole pipeline (for slope-based HW timing)."""
    nc = bacc.Bacc(None, target_bir_lowering=False)

    # xt / y are flat [P, NDT*cap] with chunk-contiguous host layouts so
    # every DMA moves one contiguous per-partition segment (efficient
    # descriptors).  xt chunk s holds [NDT, sz_s] at flat offset NDT*off_s;
    # y is dt2-major: y[:, dt2*cap + tok].
    xt = nc.declare_dram_parameter("xt", [P, NDT * cap], bf16, isOutput=False)
    w1 = nc.declare_dram_parameter("w1", [P, NFT, NDT, P], bf16, isOutput=False)
    w2 = nc.declare_dram_parameter("w2", [P, NDT, NFT, P], bf16, isOutput=False)
    b1 = nc.declare_dram_parameter("b1", [P, NFT], f32, isOutput=False)
    b2 = nc.declare_dram_parameter("b2", [P, NDT], f32, isOutput=False)
    y = nc.declare_dram_parameter("y", [P, NDT * cap], bf16, isOutput=True)

    subs = _subs(cap)

    with tile.TileContext(nc) as tc:
        with (
            tc.tile_pool(name="const", bufs=1) as const_pool,
            tc.tile_pool(name="xt", bufs=1) as xt_pool,
            tc.tile_pool(name="h", bufs=1) as h_pool,
            tc.tile_pool(name="w1", bufs=3) as w1_pool,
            tc.tile_pool(name="w2", bufs=2) as w2_pool,
            tc.tile_pool(name="yo", bufs=1) as y_pool,
            tc.tile_pool(name="ps1", bufs=4, space="PSUM") as ps1_pool,
            tc.tile_pool(name="ps2", bufs=4, space="PSUM") as ps2_pool,
        ):
            b1_sb = const_pool.tile([P, NFT], f32, name="b1sb")
            b2_sb = const_pool.tile([P, NDT], f32, name="b2sb")

            for rep in range(repeats):
                # DMA startup order matters: the first weight tile must not
                # queue behind the 4.5MB xt bulk.  Sync ring carries the
                # weight stream (w1t0 first) plus every other xt chunk
                # interleaved after the early weight tiles; the Scalar ring
                # carries the remaining xt chunks (and, in phase 2, y-out).
                xt_sb = xt_pool.tile([P, NDT * cap], bf16, name="xts")
                h_sb = h_pool.tile([P, NFT, cap], bf16, name="hsb")
                w1_tiles = {}
                w1_tiles[0] = w1_pool.tile([P, NDT, P], bf16, name="w1t")
                nc.sync.dma_start(out=w1_tiles[0][:], in_=w1[:, ts(0, 1)])
                if rep == 0:
                    nc.sync.dma_start(out=b1_sb[:], in_=b1[:])
                    nc.sync.dma_start(out=b2_sb[:], in_=b2[:])
                for i, (off, sz) in enumerate(subs):
                    eng = nc.scalar if i % 2 == 0 else nc.sync
                    if i % 2 == 1:
                        # prefetch the next weight tile between sync-ring
                        # xt chunks so the weight stream stays ahead
                        ftn = (i + 1) // 2
                        if ftn < NFT and ftn not in w1_tiles:
                            w1_tiles[ftn] = w1_pool.tile(
                                [P, NDT, P], bf16, name="w1t"
                            )
                            nc.sync.dma_start(
                                out=w1_tiles[ftn][:], in_=w1[:, ts(ftn, 1)]
                            )
                    eng.dma_start(
                        out=xt_sb[:, ds(NDT * off, NDT * sz)],
                        in_=xt[:, ds(NDT * off, NDT * sz)],
                    )

                if rep == 0:
                    # Pre-warm the PE while the xt bulk DMA lands: ~32 dummy
                    # matmuls (~3.4us busy) release the HAM clock-gate to
                    # 2.4GHz before the real stream starts.  Depends only on
                    # w1t0 (first weight tile, ~1.5us).
                    w0t = w1_tiles[0]
                    for wi in range(32):
                        psw = ps1_pool.tile([P, P], f32, name="ps1")
                        nc.tensor.matmul(
                            psw[:],
                            w0t[:, ts(0, 1)].squeeze(),
                            w0t[:, ts(1, 1)].squeeze(),
                            start=True,
                            stop=True,
                        )

                # ---- Phase 1: h = gelu(X @ W1 + b1), W1 streamed once ----
                for ft in range(NFT):
                    if ft in w1_tiles:
                        w1t = w1_tiles[ft]
                    else:
                        w1t = w1_pool.tile([P, NDT, P], bf16, name="w1t")
                        nc.sync.dma_start(out=w1t[:], in_=w1[:, ts(ft, 1)])
                    for off, sz in subs:
                        ps1 = ps1_pool.tile([P, sz], f32, name="ps1")
                        for dt in range(NDT):
                            nc.tensor.matmul(
                                ps1[:],
                                w1t[:, ts(dt, 1)].squeeze(),
                                xt_sb[:, ds(NDT * off + dt * sz, sz)],
                                start=(dt == 0),
                                stop=(dt == NDT - 1),
                            )
                        nc.scalar.activation(
                            h_sb[:, ts(ft, 1), ds(off, sz)].squeeze(),
                            ps1[:],
                            mybir.ActivationFunctionType.Gelu,
                            bias=b1_sb[:, ts(ft, 1)],
                        )

                # ---- Phase 2: y = h @ W2 + b2, W2 streamed once ----
                for dt2 in range(NDT):
                    w2t = w2_pool.tile([P, NFT, P], bf16, name="w2t")
                    nc.sync.dma_start(out=w2t[:], in_=w2[:, ts(dt2, 1)])
                    y_sb = y_pool.tile([P, cap], bf16, name="ysb")
                    for off, sz in subs:
                        ps2 = ps2_pool.tile([P, sz], f32, name="ps2")
                        for ft in range(NFT):
                            nc.tensor.matmul(
                                ps2[:],
                                w2t[:, ts(ft, 1)].squeeze(),
                                h_sb[:, ts(ft, 1), ds(off, sz)].squeeze(),
                                start=(ft == 0),
                                stop=(ft == NFT - 1),
                            )
                        nc.vector.tensor_scalar_add(
                            y_sb[:, ds(off, sz)],
                            ps2[:],
                            b2_sb[:, ts(dt2, 1)],
                        )
                        nc.scalar.dma_start(
                            out=y[:, ds(dt2 * cap + off, sz)],
                            in_=y_sb[:, ds(off, sz)],
                        )

    nc.compile()
    return nc


def _get_built(cap: int, repeats: int = 1):
    key = (cap, repeats)
    if key not in _BUILT:
        _BUILT[key] = _build(cap, repeats)
    return _BUILT[key]


def _route(x_flat, Wr, br):
    """Router: softmax over experts, top-2, renormalized. Pure numpy."""
    logits = x_flat.astype(np.float32) @ Wr.astype(np.float32) + br.astype(np.float32)
    m = logits.max(axis=-1, keepdims=True)
    p = np.exp(logits - m)
    p /= p.sum(axis=-1, keepdims=True)
    i0 = np.argmax(p, axis=-1)
    pm = p.copy()
    pm[np.arange(p.shape[0]), i0] = -np.inf
    i1 = np.argmax(pm, axis=-1)
    w0 = p[np.arange(p.shape[0]), i0]
    w1 = p[np.arange(p.shape[0]), i1]
    s = w0 + w1
    return i0, i1, w0 / s, w1 / s


def kernel(x, Wr, br, W1, b1, W2, b2, _run_kwargs=None):
    x = np.asarray(x)
    B, L, D = x.shape
    T = B * L
    x_flat = np.ascontiguousarray(x.reshape(T, D), dtype=np.float32)

    i0, i1, w0, w1c = _route(x_flat, Wr, br)

    rows_l, wts_l = [], []
    for e in range(NUM_EXPERTS):
        sel = (i0 == e) | (i1 == e)
        rows = np.nonzero(sel)[0]
        w = np.where(i0[rows] == e, w0[rows], w1c[rows]).astype(np.float32)
        rows_l.append(rows)
        wts_l.append(w)

    max_n = max(len(r) for r in rows_l)
    cap = CAP_DEFAULT
    while cap < max_n:
        cap += 128
    nc = _get_built(cap)

    subs = _subs(cap)
    in_maps = []
    for e in range(NUM_EXPERTS):
        rows = rows_l[e]
        xe = np.zeros((cap, D_MODEL), dtype=np.float32)
        xe[: len(rows)] = x_flat[rows]
        # [cap, D] -> per sub-chunk [P, NDT, sz], flat-concatenated to
        # [P, NDT*cap] so each chunk is one contiguous per-partition block.
        xeT = xe.T.reshape(NDT, P, cap)  # [NDT, P, cap]
        xtr = _bf16(
            np.concatenate(
                [
                    np.ascontiguousarray(
                        xeT[:, :, off : off + sz].transpose(1, 0, 2)
                    ).reshape(P, NDT * sz)
                    for off, sz in subs
                ],
                axis=1,
            )
        )
        w1r = _bf16(
            np.ascontiguousarray(
                np.asarray(W1[e], dtype=np.float32)
                .reshape(NDT, P, NFT, P)
                .transpose(1, 2, 0, 3)
            )
        )
        w2r = _bf16(
            np.ascontiguousarray(
                np.asarray(W2[e], dtype=np.float32)
                .reshape(NFT, P, NDT, P)
                .transpose(1, 2, 0, 3)
            )
        )
        b1r = np.ascontiguousarray(
            np.asarray(b1[e], dtype=np.float32).reshape(NFT, P).T
        )
        b2r = np.ascontiguousarray(
            np.asarray(b2[e], dtype=np.float32).reshape(NDT, P).T
        )
        in_maps.append(
            {"xt": xtr, "w1": w1r, "w2": w2r, "b1": b1r, "b2": b2r}
        )

    kw = dict(_run_kwargs or {})
    res = run_bass_kernel_spmd(nc, in_maps, list(range(NUM_EXPERTS)), **kw)

    out = np.zeros((T, D_MODEL), dtype=np.float32)
    for e in range(NUM_EXPERTS):
        rows = rows_l[e]
        ye = np.asarray(res.results[e]["y"]).astype(np.float32)  # [P, NDT*cap]
        ye = ye.reshape(P, NDT, cap).transpose(1, 0, 2).reshape(D_MODEL, cap)
        out[rows] += wts_l[e][:, None] * ye[:, : len(rows)].T

    kernel._last_result = res
    kernel._last_in_maps = in_maps
    kernel._last_cap = cap
    return out.reshape(B, L, D_MODEL)


def make_bench_runner(nc, in_maps, n_cores=NUM_EXPERTS):
    """Device-resident repeat-execution runner for timing (mirrors
    bass2jax.run_bass_via_pjrt's multi-core path, but stages inputs on
    device once and creates donated zero outputs on-device)."""
    import jax
    import jax.numpy as jnp
    from jax.experimental.shard_map import shard_map
    from jax.sharding import Mesh, NamedSharding, PartitionSpec

    from concourse import bass2jax
    from concourse import mybir as _mybir

    bass2jax.install_neuronx_cc_hook()

    part_name = (
        nc.partition_id_tensor.name if nc.partition_id_tensor else None
    )
    in_names, out_names, out_avals = [], [], []
    for alloc in nc.m.functions[0].allocations:
        if not isinstance(alloc, _mybir.MemoryLocationSet):
            continue
        name = alloc.memorylocations[0].name
        if alloc.kind == "ExternalInput":
            if name != part_name:
                in_names.append(name)
        elif alloc.kind == "ExternalOutput":
            out_names.append(name)
            out_avals.append(
                jax.core.ShapedArray(
                    tuple(alloc.tensor_shape), _mybir.dt.np(alloc.dtype)
                )
            )
    n_params = len(in_names)
    all_in = in_names + out_names
    if part_name is not None:
        all_in = all_in + [part_name]

    def _body(*args):
        operands = list(args)
        if part_name is not None:
            operands.append(bass2jax.partition_id_tensor())
        outs = bass2jax._bass_exec_p.bind(
            *operands,
            out_avals=tuple(out_avals),
            in_names=tuple(all_in),
            out_names=tuple(out_names),
            lowering_input_output_aliases=(),
            sim_require_finite=True,
            sim_require_nnan=True,
            nc=nc,
        )
        return tuple(outs)

    devices = jax.devices()[:n_cores]
    mesh = Mesh(np.asarray(devices), ("core",))
    spec = NamedSharding(mesh, PartitionSpec("core"))
    donate = tuple(range(n_params, n_params + len(out_names)))
    sharded = jax.jit(
        shard_map(
            _body,
            mesh=mesh,
            in_specs=(PartitionSpec("core"),) * (n_params + len(out_names)),
            out_specs=(PartitionSpec("core"),) * len(out_names),
            check_rep=False,
        ),
        donate_argnums=donate,
        keep_unused=True,
    )
    din = [
        jax.device_put(
            np.concatenate([m[name] for m in in_maps], axis=0), spec
        )
        for name in in_names
    ]
    zero_shapes = [
        (n_cores * a.shape[0], *a.shape[1:]) for a in out_avals
    ]
    zeros_fn = jax.jit(
        lambda: tuple(
            jnp.zeros(s, a.dtype) for s, a in zip(zero_shapes, out_avals)
        ),
        out_shardings=tuple(spec for _ in out_avals),
    )

    def run_once():
        return sharded(*din, *zeros_fn())

    def zeros_only():
        return zeros_fn()

    return run_once, zeros_only
